# revision 13
# baseline (speedup 1.0000x reference)
"""Trainium2 Bass kernel: per-pixel channel shuffle + 3x3 conv (stride 1, pad 1).

Problem: x [32,256,56,56] f32, w [256,256,3,3] f32 (OIHW), perm [3136,256] i32;
out[b,:,h,w] = conv3x3(xs)[b,:,h,w] where xs[b,:,l] = x[b, perm[l,:], l].

Strategy (8 NeuronCores, data-parallel over batch, 4 batches/core):
  host: pre-transpose x to pixel-major bf16 [B,3136,256]; build inverse-perm
        int16 index table in the GPSIMD local_scatter layout; pre-transform w
        with the Winograd F(2,3) row filter G into 48 [128,128] bf16 lhsT
        tiles (V[r,n] = sum_m G[r,m] w[:,:,m,n]).
  device, per batch (pipelined; work on a quarter-image starts as soon as its
  scatter tiles land, so PE never idles on the shuffle):
    1. Per 112-pixel tile (2 image rows): DMA [l, c] tile (contiguous 512B
       runs), GPSIMD local_scatter applies each pixel's inverse channel
       permutation, PE-transpose -> [c, l] in PSUM, DVE-copy rows into one of
       4 zero-padded overlapping quarter images (16 rows x 58 each, 1-row
       halos).
    2. Row-Winograd conv per quarter: DVE computes T_r = B^T-row-combos of
       the quarter (4 tensors of 7 tile-rows x 58); for each oc-tile and r,
       accumulate 6 matmuls (3 col-shifts x 2 ic-tiles, N=406) into PSUM
       M_r; DVE computes z0 = M0+M1+M2, z1 = M1-M2-M3 (the A^T combos, which
       replace plain PSUM eviction) and DMAs the even/odd output rows out
       (scalar queue) with padding-stripping nested APs.
  This does 12 matmul passes per 2 output rows instead of 18 (2/3 PE work).
"""

import os
import sys
import types
import numpy as np

_STATE = {}
LAST_RESULT = None

B, C, H, W = 32, 256, 56, 56
HW = H * W
TL = 112          # pixels per scatter tile (2 image rows)
NT = 28           # scatter tiles per image
NQ = 4            # quarter images
QW = 58           # padded row width
QCT = 16 * QW + 2  # per-ic-tile span in a quarter tile (+2 overrun slack)
NG = 406          # matmul free size = 7 tile-rows x 58
TSPAN = 408       # per-r span in the T tensor (+2 shift slack)
N_CORES = 8
B_LOC = B // N_CORES


def _install_ntff_shim():
    # antenv.axon_hooks is absent in some images; provide it so trace=True
    # (BASS_TRACE=1) can capture NTFF profiles instead of crashing.
    name = "antenv.axon_hooks"
    if name in sys.modules:
        return
    try:
        import antenv  # noqa: F401

        m = types.ModuleType(name)
        m._hook = None
        m.set_axon_ntff_profile_hook = lambda h: setattr(m, "_hook", h)
        m.get_axon_ntff_profile_hook = lambda: m._hook
        sys.modules[name] = m
        setattr(sys.modules["antenv"], "axon_hooks", m)
        from trn_agent_boot.trn_boot import _ntff_profile_via_ctypes

        hook = _ntff_profile_via_ctypes("/opt/axon/libaxon_pjrt.so")
        if hook is not None:
            m.set_axon_ntff_profile_hook(hook)
    except Exception:
        pass


def _build_kernel():
    import concourse.bass as bass
    import concourse.mybir as mybir
    from concourse import bacc, tile
    from concourse.masks import make_identity
    from contextlib import ExitStack

    F32 = mybir.dt.float32
    BF16 = mybir.dt.bfloat16
    I16 = mybir.dt.int16

    nc = bacc.Bacc("TRN2", target_bir_lowering=False, debug=False, num_devices=N_CORES)

    xt = nc.dram_tensor("xt", [B_LOC, HW, C], BF16, kind="ExternalInput")
    wt = nc.dram_tensor("wt", [48, 128, 128], BF16, kind="ExternalInput")
    idxt = nc.dram_tensor("idxt", [128, NT * 256], I16, kind="ExternalInput")
    out = nc.dram_tensor("out", [B_LOC, C, HW], F32, kind="ExternalOutput")

    with tile.TileContext(nc) as tc, ExitStack() as ctx:
        const = ctx.enter_context(tc.tile_pool(name="const", bufs=1))
        ident = const.tile([128, 128], BF16)
        make_identity(nc, ident[:, :])

        # Pre-warm the GPSIMD local_scatter library (~6us IRAM load) with a
        # tiny all-ignored scatter so real scatters start ASAP.
        dd = const.tile([16, 256], BF16, name="dd", tag="dd")
        nc.vector.memset(dd[:, :], 0.0)
        didx = const.tile([16, 16], I16, name="didx", tag="didx")
        nc.vector.memset(didx[:, :], -1)
        dout = const.tile([16, 256], BF16, name="dout", tag="dout")
        nc.gpsimd.local_scatter(
            out_ap=dout[:, :],
            data_ap=dd[:, :],
            idxs_ap=didx[:, :],
            channels=16,
            num_elems=256,
            num_idxs=16,
        )

        idxsb = const.tile([128, NT * 256], I16)
        wsb = const.tile([128, 48 * 128], BF16)

        # 8 persistent quarter tiles (4 quarters x double buffer across
        # batches); zero only the padding borders (cols 0/57, top/bottom halo
        # rows, overrun slack) once -- interiors are overwritten every batch.
        # Quarter layout is parity-split per ic-tile: [even rows 8x58 | odd
        # rows 8x58 | 2 slack] so the Winograd row combos are contiguous ops.
        # Quarter-local row L (= image row 14q+L-1): L=2e -> even[e], L=2o+1
        # -> odd[o].
        qts = []
        for qi in range(2 * NQ):
            qt = const.tile([128, 2 * QCT], BF16, name=f"qt{qi}", tag=f"qt{qi}")
            for ct in range(2):
                for pb in (ct * QCT, ct * QCT + 8 * QW):  # even/odd plane base
                    rows = qt[:, pb : pb + 8 * QW].rearrange(
                        "p (r x) -> p r x", r=8
                    )
                    nc.vector.memset(rows[:, :, 0:1], 0.0)
                    nc.vector.memset(rows[:, :, 57:58], 0.0)
                nc.vector.memset(qt[:, ct * QCT + 16 * QW : (ct + 1) * QCT], 0.0)
                if qi % NQ == 0:  # top padding row = local row 0 = even[0]
                    nc.vector.memset(qt[:, ct * QCT : ct * QCT + QW], 0.0)
                if qi % NQ == NQ - 1:  # bottom padding = local 15 = odd[7]
                    nc.vector.memset(
                        qt[:, ct * QCT + 15 * QW : ct * QCT + 16 * QW], 0.0
                    )
            qts.append(qt)

        # 4 persistent T tensors (2 ct x double buffer over quarters); only
        # the 2-elem shift slack after each r-span needs zeroing once.
        tts = []
        for ti in range(4):
            tt = const.tile([128, 4 * TSPAN], BF16, name=f"tt{ti}", tag=f"tt{ti}")
            for r in range(4):
                nc.vector.memset(tt[:, r * TSPAN + NG : (r + 1) * TSPAN], 0.0)
            tts.append(tt)

        # input DMAs all on the sync queue, interleaved by the time they are
        # needed; weights after the first few x tiles
        nc.sync.dma_start(out=idxsb[:, 0:512], in_=idxt[:, 0:512])

        xin_pool = ctx.enter_context(tc.tile_pool(name="xin", bufs=12))
        sout_pool = ctx.enter_context(tc.tile_pool(name="sout", bufs=12))
        tmp_pool = ctx.enter_context(tc.tile_pool(name="tmp", bufs=10))
        outst_pool = ctx.enter_context(tc.tile_pool(name="outst", bufs=4))
        psb_pool = ctx.enter_context(tc.tile_pool(name="psb", bufs=1, space="PSUM"))
        psbig = psb_pool.tile([128, 1024], BF16)  # 1 bank, 4 ps2 slots of 224
        mpsum_pool = ctx.enter_context(tc.tile_pool(name="mpsum", bufs=7, space="PSUM"))

        def shuffle_tile(b, t):
            # 2 image rows (2t, 2t+1) -> scatter -> [c, l] -> quarter tiles
            xin = xin_pool.tile([128, 256], BF16, name="xin", tag="xin")
            nc.sync.dma_start(
                out=xin[0:TL, :], in_=xt[b, t * TL : (t + 1) * TL, :]
            )
            if b == 0 and t == 0:
                nc.sync.dma_start(out=idxsb[:, 512 : 14 * 256], in_=idxt[:, 512 : 14 * 256])
            if b == 0 and t == 1:
                nc.sync.dma_start(
                    out=wsb[:, :],
                    in_=bass.AP(wt, 0, [[128, 128], [128 * 128, 48], [1, 128]]),
                )
            if b == 0 and t == 2:
                nc.sync.dma_start(
                    out=idxsb[:, 14 * 256 :], in_=idxt[:, 14 * 256 :]
                )
            sout = sout_pool.tile([128, 256], BF16, name="sout", tag="sout")
            nc.gpsimd.local_scatter(
                out_ap=sout[0:TL, :],
                data_ap=xin[0:TL, :],
                idxs_ap=idxsb[0:TL, t * 256 : (t + 1) * 256],
                channels=TL,
                num_elems=256,
                num_idxs=256,
            )
            sl = (t % 4) * 224
            ps2 = psbig[:, sl : sl + 224]
            for ct in range(2):
                nc.tensor.transpose(
                    ps2[:, ct * TL : (ct + 1) * TL],
                    sout[0:TL, ct * 128 : (ct + 1) * 128],
                    ident[0:TL, 0:TL],
                )
            q, i = divmod(t, 7)  # main quarter, local 2-row index
            qt = qts[(b % 2) * NQ + q]
            for ct in range(2):
                eb = ct * QCT
                ob = ct * QCT + 8 * QW
                # image row 2t = quarter-local row 2i+1 = odd[i];
                # image row 2t+1 = local row 2i+2 = even[i+1]
                nc.vector.tensor_copy(
                    qt[:, ob + i * QW + 1 : ob + i * QW + 57],
                    ps2[:, ct * TL : ct * TL + 56],
                )
                nc.vector.tensor_copy(
                    qt[:, eb + (i + 1) * QW + 1 : eb + (i + 1) * QW + 57],
                    ps2[:, ct * TL + 56 : ct * TL + 112],
                )
                if i == 0 and q > 0:
                    # row 2t is also the trailing halo (local 15 = odd[7]) of q-1
                    qprev = qts[(b % 2) * NQ + q - 1]
                    nc.vector.tensor_copy(
                        qprev[:, ob + 7 * QW + 1 : ob + 7 * QW + 57],
                        ps2[:, ct * TL : ct * TL + 56],
                    )
                if i == 6 and q < NQ - 1:
                    # row 2t+1 is also the leading halo (local 0 = even[0]) of q+1
                    qnext = qts[(b % 2) * NQ + q + 1]
                    nc.vector.tensor_copy(
                        qnext[:, eb + 1 : eb + 57],
                        ps2[:, ct * TL + 56 : ct * TL + 112],
                    )

        def transform(b, q):
            # B^T row combos over d_rho = local row 2*tau + rho, for all
            # tau at once -- contiguous thanks to the parity-split layout:
            # d0 = even[0:7], d1 = odd[0:7], d2 = even[1:8], d3 = odd[1:8].
            qt = qts[(b % 2) * NQ + q]
            for ct in range(2):
                eb = ct * QCT
                ob = ct * QCT + 8 * QW
                e0 = qt[:, eb : eb + NG]
                e1 = qt[:, eb + QW : eb + QW + NG]
                o0 = qt[:, ob : ob + NG]
                o1 = qt[:, ob + QW : ob + QW + NG]
                tt = tts[(q % 2) * 2 + ct]
                nc.vector.tensor_sub(tt[:, 0 * TSPAN : 0 * TSPAN + NG], e0, e1)
                nc.vector.tensor_add(tt[:, 1 * TSPAN : 1 * TSPAN + NG], o0, e1)
                nc.vector.tensor_sub(tt[:, 2 * TSPAN : 2 * TSPAN + NG], e1, o0)
                nc.vector.tensor_sub(tt[:, 3 * TSPAN : 3 * TSPAN + NG], o0, o1)

        # Deferred DVE work (A^T combos): issued between later shuffle tiles
        # so these matmul-dependent ops never block pipeline-critical copies
        # in the in-order DVE queue.
        pending = []

        def drain_pending(k):
            for _ in range(min(k, len(pending))):
                pending.pop(0)()

        def gemm_quarter(b, q):
            for oct in range(2):
                out5 = out[b, oct * 128 : (oct + 1) * 128, :].rearrange(
                    "p (a u x) -> p a u x", a=28, u=2
                )
                Ms = []
                for r in range(4):
                    mp = mpsum_pool.tile([128, NG], F32, name="mp", tag="mp")
                    Ms.append(mp)
                    for ct in range(2):
                        tt = tts[(q % 2) * 2 + ct]
                        for n in range(3):
                            widx = ((r * 3 + n) * 2 + ct) * 2 + oct
                            nc.tensor.matmul(
                                mp[:, :],
                                lhsT=wsb[:, widx * 128 : (widx + 1) * 128],
                                rhs=tt[:, r * TSPAN + n : r * TSPAN + n + NG],
                                start=(ct == 0 and n == 0),
                                stop=(ct == 1 and n == 2),
                            )
                # A^T combos: z0 = M0+M1+M2 (even rows), z1 = M1-M2-M3 (odd).
                # DVE reads only one PSUM operand per op, so M0/M1 are staged
                # to SBUF via the otherwise-idle scalar engine (issued inline;
                # the scalar queue has nothing pipeline-critical behind it).
                e0 = tmp_pool.tile([128, NG], F32, name="e0", tag="tmp")
                nc.scalar.copy(e0[:, :], Ms[0][:, :])
                e1 = tmp_pool.tile([128, NG], F32, name="e1", tag="tmp")
                nc.scalar.copy(e1[:, :], Ms[1][:, :])

                def combos(Ms=Ms, e0=e0, e1=e1, out5=out5, q=q):
                    s01 = tmp_pool.tile([128, NG], F32, name="s01", tag="tmp")
                    nc.vector.tensor_add(s01[:, :], e0[:, :], Ms[1][:, :])
                    z0 = outst_pool.tile([128, NG], F32, name="z0", tag="ost")
                    nc.vector.tensor_add(z0[:, :], s01[:, :], Ms[2][:, :])
                    nc.scalar.dma_start(
                        out=out5[:, 7 * q : 7 * q + 7, 0:1, :],
                        in_=z0[:, :].rearrange("p (a u x) -> p a u x", a=7, u=1)[
                            :, :, :, 0:56
                        ],
                    )
                    d12 = tmp_pool.tile([128, NG], F32, name="d12", tag="tmp")
                    nc.vector.tensor_sub(d12[:, :], e1[:, :], Ms[2][:, :])
                    z1 = outst_pool.tile([128, NG], F32, name="z1", tag="ost")
                    nc.vector.tensor_sub(z1[:, :], d12[:, :], Ms[3][:, :])
                    nc.scalar.dma_start(
                        out=out5[:, 7 * q : 7 * q + 7, 1:2, :],
                        in_=z1[:, :].rearrange("p (a u x) -> p a u x", a=7, u=1)[
                            :, :, :, 0:56
                        ],
                    )

                pending.append(combos)

        for b in range(B_LOC):
            if b > 0:
                transform(b - 1, 3)
                gemm_quarter(b - 1, 3)
            for t in range(NT):
                shuffle_tile(b, t)
                drain_pending(1)
                if t in (8, 15, 22):
                    q = (t - 8) // 7
                    transform(b, q)
                    gemm_quarter(b, q)
        transform(B_LOC - 1, 3)
        gemm_quarter(B_LOC - 1, 3)
        drain_pending(len(pending))

    nc.compile()
    return nc


def _host_prep(x, w, perm):
    import ml_dtypes

    # pixel-major bf16: [B, HW, C]
    xf = np.ascontiguousarray(
        x.reshape(B, C, HW).transpose(0, 2, 1)
    ).astype(ml_dtypes.bfloat16)

    # Winograd F(2,3) row-filter transform: V[r,n] = sum_m G[r,m] w[:,:,m,n]
    wf = np.asarray(w, dtype=np.float64)
    G = np.array([[1, 0, 0], [0.5, 0.5, 0.5], [0.5, -0.5, 0.5], [0, 0, 1]])
    V = np.einsum("rm,ocmn->rnoc", G, wf)  # [4, 3, OC, C]
    wt = np.empty((48, 128, 128), dtype=ml_dtypes.bfloat16)
    for r in range(4):
        for n in range(3):
            for ct in range(2):
                for oct in range(2):
                    i = ((r * 3 + n) * 2 + ct) * 2 + oct
                    wt[i] = (
                        V[r, n, oct * 128 : (oct + 1) * 128, ct * 128 : (ct + 1) * 128]
                        .T.astype(ml_dtypes.bfloat16)
                    )

    iperm = np.empty((HW, C), dtype=np.int16)
    np.put_along_axis(
        iperm, perm.astype(np.int64), np.arange(C, dtype=np.int16)[None, :], axis=1
    )
    idxt = np.zeros((128, NT * 256), dtype=np.int16)
    for t in range(NT):
        idxt[0:TL, t * 256 : (t + 1) * 256] = iperm[t * TL : t * TL + TL, :]

    in_maps = []
    for cidx in range(N_CORES):
        in_maps.append(
            {
                "xt": np.ascontiguousarray(xf[cidx * B_LOC : (cidx + 1) * B_LOC]),
                "wt": wt,
                "idxt": idxt,
            }
        )
    return in_maps


def kernel(x, w, perm):
    global LAST_RESULT
    _install_ntff_shim()
    from concourse.bass_utils import run_bass_kernel_spmd

    x = np.asarray(x, dtype=np.float32)
    w = np.asarray(w, dtype=np.float32)
    perm = np.asarray(perm)

    if "nc" not in _STATE:
        _STATE["nc"] = _build_kernel()
    nc = _STATE["nc"]

    in_maps = _host_prep(x, w, perm)
    res = run_bass_kernel_spmd(nc, in_maps, core_ids=list(range(N_CORES)))
    LAST_RESULT = res
    out = np.concatenate(
        [r["out"].reshape(B_LOC, C, H, W) for r in res.results], axis=0
    )
    return out.astype(np.float32)


# revision 14
# speedup vs baseline: 1.5397x; 1.5397x over previous
"""Trainium2 Bass kernel: per-pixel channel shuffle + 3x3 conv (stride 1, pad 1).

Problem: x [32,256,56,56] f32, w [256,256,3,3] f32 (OIHW), perm [3136,256] i32;
out[b,:,h,w] = conv3x3(xs)[b,:,h,w] where xs[b,:,l] = x[b, perm[l,:], l].

Strategy (8 NeuronCores, data-parallel over batch, 4 batches/core):
  host: pre-transpose x to pixel-major bf16 [B,3136,256]; build inverse-perm
        int16 index table in the GPSIMD local_scatter layout; pre-transpose w
        into 36 [128,128] bf16 lhsT tiles.
  device, per batch (pipelined; conv of a quarter-image starts as soon as its
  scatter tiles land, so PE never idles on the shuffle):
    1. Per 112-pixel tile (2 image rows): DMA [l, c] tile (contiguous 512B
       runs), GPSIMD local_scatter applies each pixel's inverse channel
       permutation, PE-transpose -> [c, l] in PSUM, DVE-copy rows into one of
       4 zero-padded overlapping quarter images (16 rows x 58 each, 1-row
       halos).
    2. Conv as implicit GEMM per quarter: for 2 oc-tiles x 2 row-groups
       (N=406 = 7 padded rows), accumulate 18 matmuls (9 taps x 2 ic-tiles)
       into PSUM (f32), evict via DVE, DMA out (scalar queue) with a [7,56]
       nested AP that strips the padding.
"""

import os
import sys
import types
import numpy as np

_STATE = {}
LAST_RESULT = None

B, C, H, W = 32, 256, 56, 56
HW = H * W
TL = 112          # pixels per scatter tile (2 image rows)
NT = 28           # scatter tiles per image
NQ = 4            # quarter images
QROWS = 16        # rows per quarter incl. 1-row halo each side (14 + 2)
QW = 58           # padded row width
QCT = QROWS * QW + 2   # per-ic-tile span in a quarter tile (+2 overrun slack)
NG = 406          # conv group free size = 7 rows x 58
N_CORES = 8
B_LOC = B // N_CORES


def _install_ntff_shim():
    # antenv.axon_hooks is absent in some images; provide it so trace=True
    # (BASS_TRACE=1) can capture NTFF profiles instead of crashing.
    name = "antenv.axon_hooks"
    if name in sys.modules:
        return
    try:
        import antenv  # noqa: F401

        m = types.ModuleType(name)
        m._hook = None
        m.set_axon_ntff_profile_hook = lambda h: setattr(m, "_hook", h)
        m.get_axon_ntff_profile_hook = lambda: m._hook
        sys.modules[name] = m
        setattr(sys.modules["antenv"], "axon_hooks", m)
        from trn_agent_boot.trn_boot import _ntff_profile_via_ctypes

        hook = _ntff_profile_via_ctypes("/opt/axon/libaxon_pjrt.so")
        if hook is not None:
            m.set_axon_ntff_profile_hook(hook)
    except Exception:
        pass


def _build_kernel():
    import concourse.bass as bass
    import concourse.mybir as mybir
    from concourse import bacc, tile
    from concourse.masks import make_identity
    from contextlib import ExitStack

    F32 = mybir.dt.float32
    BF16 = mybir.dt.bfloat16
    I16 = mybir.dt.int16

    nc = bacc.Bacc("TRN2", target_bir_lowering=False, debug=False, num_devices=N_CORES)

    xt = nc.dram_tensor("xt", [B_LOC, HW, C], BF16, kind="ExternalInput")
    wt = nc.dram_tensor("wt", [36, 128, 128], BF16, kind="ExternalInput")
    idxt = nc.dram_tensor("idxt", [128, NT * 256], I16, kind="ExternalInput")
    out = nc.dram_tensor("out", [B_LOC, C, HW], F32, kind="ExternalOutput")

    with tile.TileContext(nc) as tc, ExitStack() as ctx:
        const = ctx.enter_context(tc.tile_pool(name="const", bufs=1))
        ident = const.tile([128, 128], BF16)
        make_identity(nc, ident[:, :])

        # Pre-warm the GPSIMD local_scatter library (~6us IRAM load) with a
        # tiny all-ignored scatter so real scatters start ASAP.
        dd = const.tile([16, 256], BF16, name="dd", tag="dd")
        nc.vector.memset(dd[:, :], 0.0)
        didx = const.tile([16, 16], I16, name="didx", tag="didx")
        nc.vector.memset(didx[:, :], -1)
        dout = const.tile([16, 256], BF16, name="dout", tag="dout")
        nc.gpsimd.local_scatter(
            out_ap=dout[:, :],
            data_ap=dd[:, :],
            idxs_ap=didx[:, :],
            channels=16,
            num_elems=256,
            num_idxs=16,
        )

        idxsb = const.tile([128, NT * 256], I16)
        wsb = const.tile([128, 36 * 128], BF16)

        # 8 persistent quarter tiles (4 quarters x double buffer across
        # batches); zero only the padding borders (cols 0/57, top/bottom halo
        # rows, overrun slack) once -- interiors are overwritten every batch.
        qts = []
        for qi in range(2 * NQ):
            qt = const.tile([128, 2 * QCT], BF16, name=f"qt{qi}", tag=f"qt{qi}")
            for ct in range(2):
                base = ct * QCT
                rows = qt[:, base : base + 16 * QW].rearrange(
                    "p (r x) -> p r x", r=16
                )
                nc.vector.memset(rows[:, :, 0:1], 0.0)
                nc.vector.memset(rows[:, :, 57:58], 0.0)
                nc.vector.memset(qt[:, base + 16 * QW : base + QCT], 0.0)
                if qi % NQ == 0:
                    nc.vector.memset(qt[:, base : base + QW], 0.0)
                if qi % NQ == NQ - 1:
                    nc.vector.memset(qt[:, base + 15 * QW : base + 16 * QW], 0.0)
            qts.append(qt)

        # first small idx chunk on the sync queue (unblocks scatter tiles
        # 0-1); weights + remaining idx chunks are interleaved into the xin
        # DMA stream by the time they're needed (see shuffle_tile)
        nc.sync.dma_start(out=idxsb[:, 0:512], in_=idxt[:, 0:512])

        xin_pool = ctx.enter_context(tc.tile_pool(name="xin", bufs=12))
        sout_pool = ctx.enter_context(tc.tile_pool(name="sout", bufs=12))
        outst_pool = ctx.enter_context(tc.tile_pool(name="outst", bufs=4))
        tps_pool = ctx.enter_context(tc.tile_pool(name="tps", bufs=3, space="PSUM"))
        mpsum_pool = ctx.enter_context(tc.tile_pool(name="mpsum", bufs=4, space="PSUM"))

        def shuffle_tile(b, t):
            # 2 image rows (2t, 2t+1) -> scatter -> [c, l] -> quarter tiles
            xin = xin_pool.tile([128, 256], BF16, name="xin", tag="xin")
            nc.sync.dma_start(
                out=xin[0:TL, :], in_=xt[b, t * TL : (t + 1) * TL, :]
            )
            if b == 0 and t == 0:
                nc.sync.dma_start(
                    out=idxsb[:, 512 : 14 * 256], in_=idxt[:, 512 : 14 * 256]
                )
            if b == 0 and t == 1:
                nc.sync.dma_start(
                    out=wsb[:, :],
                    in_=bass.AP(wt, 0, [[128, 128], [128 * 128, 36], [1, 128]]),
                )
            if b == 0 and t == 2:
                nc.sync.dma_start(
                    out=idxsb[:, 14 * 256 :], in_=idxt[:, 14 * 256 :]
                )
            sout = sout_pool.tile([128, 256], BF16, name="sout", tag="sout")
            nc.gpsimd.local_scatter(
                out_ap=sout[0:TL, :],
                data_ap=xin[0:TL, :],
                idxs_ap=idxsb[0:TL, t * 256 : (t + 1) * 256],
                channels=TL,
                num_elems=256,
                num_idxs=256,
            )
            ps2 = tps_pool.tile([128, 2 * TL], BF16, name="ps2", tag="ps2")
            for ct in range(2):
                nc.tensor.transpose(
                    ps2[:, ct * TL : (ct + 1) * TL],
                    sout[0:TL, ct * 128 : (ct + 1) * 128],
                    ident[0:TL, 0:TL],
                )
            q, i = divmod(t, 7)  # main quarter, local 2-row index
            qt = qts[(b % 2) * NQ + q]
            for ct in range(2):
                # rows 2t, 2t+1 = quarter-local rows 2i+1, 2i+2
                dst = qt[
                    :, ct * QCT + (2 * i + 1) * QW : ct * QCT + (2 * i + 3) * QW
                ].rearrange("p (r x) -> p r x", r=2)[:, :, 1:57]
                src = ps2[:, ct * TL : (ct + 1) * TL].rearrange(
                    "p (r x) -> p r x", r=2
                )
                nc.vector.tensor_copy(dst, src)
                if i == 0 and q > 0:
                    # row 2t is also the trailing halo (local row 15) of q-1
                    qprev = qts[(b % 2) * NQ + q - 1]
                    nc.vector.tensor_copy(
                        qprev[:, ct * QCT + 15 * QW + 1 : ct * QCT + 15 * QW + 57],
                        ps2[:, ct * TL : ct * TL + 56],
                    )
                if i == 6 and q < NQ - 1:
                    # row 2t+1 is also the leading halo (local row 0) of q+1
                    qnext = qts[(b % 2) * NQ + q + 1]
                    nc.vector.tensor_copy(
                        qnext[:, ct * QCT + 1 : ct * QCT + 57],
                        ps2[:, ct * TL + 56 : ct * TL + 112],
                    )

        def conv_quarter(b, q):
            qt = qts[(b % 2) * NQ + q]
            for oct in range(2):
                for j in range(2):
                    mp = mpsum_pool.tile([128, NG], F32, name="mp", tag="mp")
                    for i in range(18):
                        ct, tap = divmod(i, 9)
                        dh, dw = divmod(tap, 3)
                        q0 = ct * QCT + (7 * j + dh) * QW + dw
                        widx = (ct * 9 + tap) * 2 + oct
                        nc.tensor.matmul(
                            mp[:, :],
                            lhsT=wsb[:, widx * 128 : (widx + 1) * 128],
                            rhs=qt[:, q0 : q0 + NG],
                            start=(i == 0),
                            stop=(i == 17),
                        )
                    ost = outst_pool.tile([128, NG], F32, name="ost", tag="ost")
                    nc.vector.tensor_copy(ost[:, :], mp[:, :])
                    row0 = 14 * q + 7 * j
                    nc.scalar.dma_start(
                        out=out[
                            b, oct * 128 : (oct + 1) * 128, row0 * 56 : row0 * 56 + 392
                        ],
                        in_=ost[:, :].rearrange("p (r x) -> p r x", r=7)[:, :, 0:56],
                    )

        for b in range(B_LOC):
            for t in range(NT):
                shuffle_tile(b, t)
                # quarter q is fully scattered once tile 7q+7 lands
                if t in (7, 14, 21):
                    conv_quarter(b, (t - 7) // 7)
            conv_quarter(b, 3)

    nc.compile()
    return nc


def _host_prep(x, w, perm):
    import ml_dtypes

    # pixel-major bf16: [B, HW, C]
    xf = np.ascontiguousarray(
        x.reshape(B, C, HW).transpose(0, 2, 1)
    ).astype(ml_dtypes.bfloat16)

    wt = np.empty((36, 128, 128), dtype=ml_dtypes.bfloat16)
    wf = np.asarray(w, dtype=np.float32)
    for ct in range(2):
        for tap in range(9):
            kh, kw = divmod(tap, 3)
            for oct in range(2):
                i = (ct * 9 + tap) * 2 + oct
                wt[i] = wf[
                    oct * 128 : (oct + 1) * 128, ct * 128 : (ct + 1) * 128, kh, kw
                ].T.astype(ml_dtypes.bfloat16)

    iperm = np.empty((HW, C), dtype=np.int16)
    np.put_along_axis(
        iperm, perm.astype(np.int64), np.arange(C, dtype=np.int16)[None, :], axis=1
    )
    idxt = np.zeros((128, NT * 256), dtype=np.int16)
    for t in range(NT):
        idxt[0:TL, t * 256 : (t + 1) * 256] = iperm[t * TL : t * TL + TL, :]

    in_maps = []
    for cidx in range(N_CORES):
        in_maps.append(
            {
                "xt": np.ascontiguousarray(xf[cidx * B_LOC : (cidx + 1) * B_LOC]),
                "wt": wt,
                "idxt": idxt,
            }
        )
    return in_maps


def kernel(x, w, perm):
    global LAST_RESULT
    _install_ntff_shim()
    from concourse.bass_utils import run_bass_kernel_spmd

    x = np.asarray(x, dtype=np.float32)
    w = np.asarray(w, dtype=np.float32)
    perm = np.asarray(perm)

    if "nc" not in _STATE:
        _STATE["nc"] = _build_kernel()
    nc = _STATE["nc"]

    in_maps = _host_prep(x, w, perm)
    res = run_bass_kernel_spmd(nc, in_maps, core_ids=list(range(N_CORES)))
    LAST_RESULT = res
    out = np.concatenate(
        [r["out"].reshape(B_LOC, C, H, W) for r in res.results], axis=0
    )
    return out.astype(np.float32)


# revision 16
# speedup vs baseline: 1.5648x; 1.0163x over previous
"""Trainium2 Bass kernel: per-pixel channel shuffle + 3x3 conv (stride 1, pad 1).

Problem: x [32,256,56,56] f32, w [256,256,3,3] f32 (OIHW), perm [3136,256] i32;
out[b,:,h,w] = conv3x3(xs)[b,:,h,w] where xs[b,:,l] = x[b, perm[l,:], l].

Strategy (8 NeuronCores, data-parallel over batch, 4 batches/core):
  host: pre-transpose x to pixel-major bf16 [B,3136,256]; build inverse-perm
        int16 index table in the GPSIMD local_scatter layout; pre-transpose w
        into 36 [128,128] bf16 lhsT tiles.
  device, per batch (pipelined; conv of a quarter-image starts as soon as its
  scatter tiles land, so PE never idles on the shuffle):
    1. Per 112-pixel tile (2 image rows): DMA [l, c] tile (contiguous 512B
       runs), GPSIMD local_scatter applies each pixel's inverse channel
       permutation, PE-transpose -> [c, l] in PSUM, DVE-copy rows into one of
       4 zero-padded overlapping quarter images (16 rows x 58 each, 1-row
       halos).
    2. Conv as implicit GEMM per quarter: for 2 oc-tiles x 2 row-groups
       (N=406 = 7 padded rows), accumulate 18 matmuls (9 taps x 2 ic-tiles)
       into PSUM (f32), evict via DVE, DMA out (scalar queue) with a [7,56]
       nested AP that strips the padding.
"""

import os
import sys
import types
import numpy as np

_STATE = {}
LAST_RESULT = None

B, C, H, W = 32, 256, 56, 56
HW = H * W
TL = 112          # pixels per scatter tile (2 image rows)
NT = 28           # scatter tiles per image
NQ = 4            # quarter images
QROWS = 16        # rows per quarter incl. 1-row halo each side (14 + 2)
QW = 58           # padded row width
QCT = QROWS * QW + 2   # per-ic-tile span in a quarter tile (+2 overrun slack)
NG = 406          # conv group free size = 7 rows x 58
N_CORES = 8
B_LOC = B // N_CORES


def _install_ntff_shim():
    # antenv.axon_hooks is absent in some images; provide it so trace=True
    # (BASS_TRACE=1) can capture NTFF profiles instead of crashing.
    name = "antenv.axon_hooks"
    if name in sys.modules:
        return
    try:
        import antenv  # noqa: F401

        m = types.ModuleType(name)
        m._hook = None
        m.set_axon_ntff_profile_hook = lambda h: setattr(m, "_hook", h)
        m.get_axon_ntff_profile_hook = lambda: m._hook
        sys.modules[name] = m
        setattr(sys.modules["antenv"], "axon_hooks", m)
        from trn_agent_boot.trn_boot import _ntff_profile_via_ctypes

        hook = _ntff_profile_via_ctypes("/opt/axon/libaxon_pjrt.so")
        if hook is not None:
            m.set_axon_ntff_profile_hook(hook)
    except Exception:
        pass


def _build_kernel():
    import concourse.bass as bass
    import concourse.mybir as mybir
    from concourse import bacc, tile
    from concourse.masks import make_identity
    from contextlib import ExitStack

    F32 = mybir.dt.float32
    BF16 = mybir.dt.bfloat16
    I16 = mybir.dt.int16

    nc = bacc.Bacc("TRN2", target_bir_lowering=False, debug=False, num_devices=N_CORES)

    xt = nc.dram_tensor("xt", [B_LOC, HW, C], BF16, kind="ExternalInput")
    wt = nc.dram_tensor("wt", [36, 128, 128], BF16, kind="ExternalInput")
    idxt = nc.dram_tensor("idxt", [128, NT * 256], I16, kind="ExternalInput")
    out = nc.dram_tensor("out", [B_LOC, C, HW], F32, kind="ExternalOutput")

    with tile.TileContext(nc) as tc, ExitStack() as ctx:
        const = ctx.enter_context(tc.tile_pool(name="const", bufs=1))
        ident = const.tile([128, 128], BF16)
        make_identity(nc, ident[:, :])

        # Pre-warm the GPSIMD local_scatter library (~6us IRAM load) with a
        # tiny all-ignored scatter so real scatters start ASAP.
        dd = const.tile([16, 256], BF16, name="dd", tag="dd")
        nc.vector.memset(dd[:, :], 0.0)
        didx = const.tile([16, 16], I16, name="didx", tag="didx")
        nc.vector.memset(didx[:, :], -1)
        dout = const.tile([16, 256], BF16, name="dout", tag="dout")
        nc.gpsimd.local_scatter(
            out_ap=dout[:, :],
            data_ap=dd[:, :],
            idxs_ap=didx[:, :],
            channels=16,
            num_elems=256,
            num_idxs=16,
        )

        idxsb = const.tile([128, NT * 256], I16)
        wsb = const.tile([128, 36 * 128], BF16)

        # 8 persistent quarter tiles (4 quarters x double buffer across
        # batches); zero only the padding borders (cols 0/57, top/bottom halo
        # rows, overrun slack) once -- interiors are overwritten every batch.
        qts = []
        for qi in range(2 * NQ):
            qt = const.tile([128, 2 * QCT], BF16, name=f"qt{qi}", tag=f"qt{qi}")
            for ct in range(2):
                base = ct * QCT
                rows = qt[:, base : base + 16 * QW].rearrange(
                    "p (r x) -> p r x", r=16
                )
                nc.vector.memset(rows[:, :, 0:1], 0.0)
                nc.vector.memset(rows[:, :, 57:58], 0.0)
                nc.vector.memset(qt[:, base + 16 * QW : base + QCT], 0.0)
                if qi % NQ == 0:
                    nc.vector.memset(qt[:, base : base + QW], 0.0)
                if qi % NQ == NQ - 1:
                    nc.vector.memset(qt[:, base + 15 * QW : base + 16 * QW], 0.0)
            qts.append(qt)

        # first small idx chunk on the sync queue (unblocks scatter tiles
        # 0-1); weights + remaining idx chunks are interleaved into the xin
        # DMA stream by the time they're needed (see shuffle_tile)
        nc.sync.dma_start(out=idxsb[:, 0:512], in_=idxt[:, 0:512])

        xin_pool = ctx.enter_context(tc.tile_pool(name="xin", bufs=12))
        sout_pool = ctx.enter_context(tc.tile_pool(name="sout", bufs=12))
        outst_pool = ctx.enter_context(tc.tile_pool(name="outst", bufs=4))
        tps_pool = ctx.enter_context(tc.tile_pool(name="tps", bufs=3, space="PSUM"))
        mpsum_pool = ctx.enter_context(tc.tile_pool(name="mpsum", bufs=5, space="PSUM"))

        def shuffle_tile(b, t):
            # 2 image rows (2t, 2t+1) -> scatter -> [c, l] -> quarter tiles
            xin = xin_pool.tile([128, 256], BF16, name="xin", tag="xin")
            nc.sync.dma_start(
                out=xin[0:TL, :], in_=xt[b, t * TL : (t + 1) * TL, :]
            )
            if b == 0 and t == 0:
                nc.sync.dma_start(
                    out=idxsb[:, 512 : 14 * 256], in_=idxt[:, 512 : 14 * 256]
                )
            if b == 0 and t == 1:
                nc.sync.dma_start(
                    out=wsb[:, :],
                    in_=bass.AP(wt, 0, [[128, 128], [128 * 128, 36], [1, 128]]),
                )
            if b == 0 and t == 2:
                nc.sync.dma_start(
                    out=idxsb[:, 14 * 256 :], in_=idxt[:, 14 * 256 :]
                )
            sout = sout_pool.tile([128, 256], BF16, name="sout", tag="sout")
            nc.gpsimd.local_scatter(
                out_ap=sout[0:TL, :],
                data_ap=xin[0:TL, :],
                idxs_ap=idxsb[0:TL, t * 256 : (t + 1) * 256],
                channels=TL,
                num_elems=256,
                num_idxs=256,
            )
            ps2 = tps_pool.tile([128, 2 * TL], BF16, name="ps2", tag="ps2")
            for ct in range(2):
                nc.tensor.transpose(
                    ps2[:, ct * TL : (ct + 1) * TL],
                    sout[0:TL, ct * 128 : (ct + 1) * 128],
                    ident[0:TL, 0:TL],
                )
            q, i = divmod(t, 7)  # main quarter, local 2-row index
            qt = qts[(b % 2) * NQ + q]
            for ct in range(2):
                # rows 2t, 2t+1 = quarter-local rows 2i+1, 2i+2
                dst = qt[
                    :, ct * QCT + (2 * i + 1) * QW : ct * QCT + (2 * i + 3) * QW
                ].rearrange("p (r x) -> p r x", r=2)[:, :, 1:57]
                src = ps2[:, ct * TL : (ct + 1) * TL].rearrange(
                    "p (r x) -> p r x", r=2
                )
                nc.vector.tensor_copy(dst, src)
                if i == 0 and q > 0:
                    # row 2t is also the trailing halo (local row 15) of q-1
                    qprev = qts[(b % 2) * NQ + q - 1]
                    nc.vector.tensor_copy(
                        qprev[:, ct * QCT + 15 * QW + 1 : ct * QCT + 15 * QW + 57],
                        ps2[:, ct * TL : ct * TL + 56],
                    )
                if i == 6 and q < NQ - 1:
                    # row 2t+1 is also the leading halo (local row 0) of q+1
                    qnext = qts[(b % 2) * NQ + q + 1]
                    nc.vector.tensor_copy(
                        qnext[:, ct * QCT + 1 : ct * QCT + 57],
                        ps2[:, ct * TL + 56 : ct * TL + 112],
                    )

        def conv_quarter(b, q):
            qt = qts[(b % 2) * NQ + q]
            for oct in range(2):
                for j in range(2):
                    mp = mpsum_pool.tile([128, NG], F32, name="mp", tag="mp")
                    for i in range(18):
                        ct, tap = divmod(i, 9)
                        dh, dw = divmod(tap, 3)
                        q0 = ct * QCT + (7 * j + dh) * QW + dw
                        widx = (ct * 9 + tap) * 2 + oct
                        nc.tensor.matmul(
                            mp[:, :],
                            lhsT=wsb[:, widx * 128 : (widx + 1) * 128],
                            rhs=qt[:, q0 : q0 + NG],
                            start=(i == 0),
                            stop=(i == 17),
                        )
                    ost = outst_pool.tile([128, NG], F32, name="ost", tag="ost")
                    nc.vector.tensor_copy(ost[:, :], mp[:, :])
                    row0 = 14 * q + 7 * j
                    nc.scalar.dma_start(
                        out=out[
                            b, oct * 128 : (oct + 1) * 128, row0 * 56 : row0 * 56 + 392
                        ],
                        in_=ost[:, :].rearrange("p (r x) -> p r x", r=7)[:, :, 0:56],
                    )

        for b in range(B_LOC):
            for t in range(NT):
                shuffle_tile(b, t)
                # quarter q is fully scattered once tile 7q+7 lands; issue
                # its conv 2 tiles later so the interleaved transposes cover
                # the DVE copy-chain latency at burst start
                if t in (9, 16, 23):
                    conv_quarter(b, (t - 9) // 7)
            conv_quarter(b, 3)

    nc.compile()
    return nc


def _host_prep(x, w, perm):
    import ml_dtypes

    # pixel-major bf16: [B, HW, C]
    xf = np.ascontiguousarray(
        x.reshape(B, C, HW).transpose(0, 2, 1)
    ).astype(ml_dtypes.bfloat16)

    wt = np.empty((36, 128, 128), dtype=ml_dtypes.bfloat16)
    wf = np.asarray(w, dtype=np.float32)
    for ct in range(2):
        for tap in range(9):
            kh, kw = divmod(tap, 3)
            for oct in range(2):
                i = (ct * 9 + tap) * 2 + oct
                wt[i] = wf[
                    oct * 128 : (oct + 1) * 128, ct * 128 : (ct + 1) * 128, kh, kw
                ].T.astype(ml_dtypes.bfloat16)

    iperm = np.empty((HW, C), dtype=np.int16)
    np.put_along_axis(
        iperm, perm.astype(np.int64), np.arange(C, dtype=np.int16)[None, :], axis=1
    )
    idxt = np.zeros((128, NT * 256), dtype=np.int16)
    for t in range(NT):
        idxt[0:TL, t * 256 : (t + 1) * 256] = iperm[t * TL : t * TL + TL, :]

    in_maps = []
    for cidx in range(N_CORES):
        in_maps.append(
            {
                "xt": np.ascontiguousarray(xf[cidx * B_LOC : (cidx + 1) * B_LOC]),
                "wt": wt,
                "idxt": idxt,
            }
        )
    return in_maps


def kernel(x, w, perm):
    global LAST_RESULT
    _install_ntff_shim()
    from concourse.bass_utils import run_bass_kernel_spmd

    x = np.asarray(x, dtype=np.float32)
    w = np.asarray(w, dtype=np.float32)
    perm = np.asarray(perm)

    if "nc" not in _STATE:
        _STATE["nc"] = _build_kernel()
    nc = _STATE["nc"]

    in_maps = _host_prep(x, w, perm)
    res = run_bass_kernel_spmd(nc, in_maps, core_ids=list(range(N_CORES)))
    LAST_RESULT = res
    out = np.concatenate(
        [r["out"].reshape(B_LOC, C, H, W) for r in res.results], axis=0
    )
    return out.astype(np.float32)


# revision 19
# speedup vs baseline: 1.6162x; 1.0328x over previous
"""Trainium2 Bass kernel: per-pixel channel shuffle + 3x3 conv (stride 1, pad 1).

Problem: x [32,256,56,56] f32, w [256,256,3,3] f32 (OIHW), perm [3136,256] i32;
out[b,:,h,w] = conv3x3(xs)[b,:,h,w] where xs[b,:,l] = x[b, perm[l,:], l].

Strategy (8 NeuronCores, data-parallel over batch, 4 batches/core):
  host: pre-transpose x to pixel-major bf16 [B,3136,256]; build inverse-perm
        int16 index table in the GPSIMD local_scatter layout; pre-transpose w
        into 36 [128,128] bf16 lhsT tiles.
  device, per batch (pipelined; conv of a quarter-image starts as soon as its
  scatter tiles land, so PE never idles on the shuffle):
    1. Per 112-pixel tile (2 image rows): DMA [l, c] tile (contiguous 512B
       runs), GPSIMD local_scatter applies each pixel's inverse channel
       permutation, PE-transpose -> [c, l] in PSUM, DVE-copy rows into one of
       4 zero-padded overlapping quarter images (16 rows x 58 each, 1-row
       halos).
    2. Conv as implicit GEMM per quarter: for 2 oc-tiles x 2 row-groups
       (N=406 = 7 padded rows), accumulate 18 matmuls (9 taps x 2 ic-tiles)
       into PSUM (f32), evict via DVE, DMA out (scalar queue) with a [7,56]
       nested AP that strips the padding.
"""

import os
import sys
import types
import numpy as np

_STATE = {}
LAST_RESULT = None

B, C, H, W = 32, 256, 56, 56
HW = H * W
TL = 112          # pixels per scatter tile (2 image rows)
NT = 28           # scatter tiles per image
NQ = 4            # quarter images
QROWS = 16        # rows per quarter incl. 1-row halo each side (14 + 2)
QW = 58           # padded row width
QCT = QROWS * QW + 2   # per-ic-tile span in a quarter tile (+2 overrun slack)
NG = 406          # conv group free size = 7 rows x 58
N_CORES = 8
B_LOC = B // N_CORES


def _install_ntff_shim():
    # antenv.axon_hooks is absent in some images; provide it so trace=True
    # (BASS_TRACE=1) can capture NTFF profiles instead of crashing.
    name = "antenv.axon_hooks"
    if name in sys.modules:
        return
    try:
        import antenv  # noqa: F401

        m = types.ModuleType(name)
        m._hook = None
        m.set_axon_ntff_profile_hook = lambda h: setattr(m, "_hook", h)
        m.get_axon_ntff_profile_hook = lambda: m._hook
        sys.modules[name] = m
        setattr(sys.modules["antenv"], "axon_hooks", m)
        from trn_agent_boot.trn_boot import _ntff_profile_via_ctypes

        hook = _ntff_profile_via_ctypes("/opt/axon/libaxon_pjrt.so")
        if hook is not None:
            m.set_axon_ntff_profile_hook(hook)
    except Exception:
        pass


def _build_kernel():
    import concourse.bass as bass
    import concourse.mybir as mybir
    from concourse import bacc, tile
    from concourse.masks import make_identity
    from contextlib import ExitStack

    F32 = mybir.dt.float32
    BF16 = mybir.dt.bfloat16
    I16 = mybir.dt.int16

    nc = bacc.Bacc("TRN2", target_bir_lowering=False, debug=False, num_devices=N_CORES)

    xt = nc.dram_tensor("xt", [B_LOC, HW, C], BF16, kind="ExternalInput")
    wt = nc.dram_tensor("wt", [36, 128, 128], BF16, kind="ExternalInput")
    idxt = nc.dram_tensor("idxt", [128, NT * 256], I16, kind="ExternalInput")
    out = nc.dram_tensor("out", [B_LOC, C, HW], F32, kind="ExternalOutput")

    with tile.TileContext(nc) as tc, ExitStack() as ctx:
        const = ctx.enter_context(tc.tile_pool(name="const", bufs=1))
        ident = const.tile([128, 128], BF16)
        make_identity(nc, ident[:, :])

        # Pre-warm the GPSIMD local_scatter library (~6us IRAM load) with a
        # tiny all-ignored scatter so real scatters start ASAP.
        dd = const.tile([16, 256], BF16, name="dd", tag="dd")
        nc.vector.memset(dd[:, :], 0.0)
        didx = const.tile([16, 16], I16, name="didx", tag="didx")
        nc.vector.memset(didx[:, :], -1)
        dout = const.tile([16, 256], BF16, name="dout", tag="dout")
        nc.gpsimd.local_scatter(
            out_ap=dout[:, :],
            data_ap=dd[:, :],
            idxs_ap=didx[:, :],
            channels=16,
            num_elems=256,
            num_idxs=16,
        )

        idxsb = const.tile([128, NT * 256], I16)
        wsb = const.tile([128, 36 * 128], BF16)

        # 8 persistent quarter tiles (4 quarters x double buffer across
        # batches); zero only the padding borders (cols 0/57, top/bottom halo
        # rows, overrun slack) once -- interiors are overwritten every batch.
        qts = []
        for qi in range(2 * NQ):
            qt = const.tile([128, 2 * QCT], BF16, name=f"qt{qi}", tag=f"qt{qi}")
            for ct in range(2):
                base = ct * QCT
                rows = qt[:, base : base + 16 * QW].rearrange(
                    "p (r x) -> p r x", r=16
                )
                nc.vector.memset(rows[:, :, 0:1], 0.0)
                nc.vector.memset(rows[:, :, 57:58], 0.0)
                nc.vector.memset(qt[:, base + 16 * QW : base + QCT], 0.0)
                if qi % NQ == 0:
                    nc.vector.memset(qt[:, base : base + QW], 0.0)
                if qi % NQ == NQ - 1:
                    nc.vector.memset(qt[:, base + 15 * QW : base + 16 * QW], 0.0)
            qts.append(qt)

        # first small idx chunk on the sync queue (unblocks scatter tiles
        # 0-1); weights + remaining idx chunks are interleaved into the xin
        # DMA stream by the time they're needed (see shuffle_tile)
        nc.sync.dma_start(out=idxsb[:, 0:512], in_=idxt[:, 0:512])

        xin_pool = ctx.enter_context(tc.tile_pool(name="xin", bufs=16))
        sout_pool = ctx.enter_context(tc.tile_pool(name="sout", bufs=16))
        outst_pool = ctx.enter_context(tc.tile_pool(name="outst", bufs=4))
        tps_pool = ctx.enter_context(tc.tile_pool(name="tps", bufs=3, space="PSUM"))
        mpsum_pool = ctx.enter_context(tc.tile_pool(name="mpsum", bufs=5, space="PSUM"))

        def shuffle_tile(b, t):
            # 2 image rows (2t, 2t+1) -> scatter -> [c, l] -> quarter tiles
            xin = xin_pool.tile([128, 256], BF16, name="xin", tag="xin")
            nc.sync.dma_start(
                out=xin[0:TL, :], in_=xt[b, t * TL : (t + 1) * TL, :]
            )
            if b == 0 and t == 0:
                nc.sync.dma_start(
                    out=idxsb[:, 512 : 14 * 256], in_=idxt[:, 512 : 14 * 256]
                )
            if b == 0 and t == 1:
                nc.sync.dma_start(
                    out=wsb[:, :],
                    in_=bass.AP(wt, 0, [[128, 128], [128 * 128, 36], [1, 128]]),
                )
            if b == 0 and t == 2:
                nc.sync.dma_start(
                    out=idxsb[:, 14 * 256 :], in_=idxt[:, 14 * 256 :]
                )
            sout = sout_pool.tile([128, 256], BF16, name="sout", tag="sout")
            nc.gpsimd.local_scatter(
                out_ap=sout[0:TL, :],
                data_ap=xin[0:TL, :],
                idxs_ap=idxsb[0:TL, t * 256 : (t + 1) * 256],
                channels=TL,
                num_elems=256,
                num_idxs=256,
            )
            ps2 = tps_pool.tile([128, 2 * TL], BF16, name="ps2", tag="ps2")
            for ct in range(2):
                nc.tensor.transpose(
                    ps2[:, ct * TL : (ct + 1) * TL],
                    sout[0:TL, ct * 128 : (ct + 1) * 128],
                    ident[0:TL, 0:TL],
                )
            q, i = divmod(t, 7)  # main quarter, local 2-row index
            qt = qts[(b % 2) * NQ + q]
            for ct in range(2):
                # rows 2t, 2t+1 = quarter-local rows 2i+1, 2i+2
                dst = qt[
                    :, ct * QCT + (2 * i + 1) * QW : ct * QCT + (2 * i + 3) * QW
                ].rearrange("p (r x) -> p r x", r=2)[:, :, 1:57]
                src = ps2[:, ct * TL : (ct + 1) * TL].rearrange(
                    "p (r x) -> p r x", r=2
                )
                nc.vector.tensor_copy(dst, src)
                if i == 0 and q > 0:
                    # row 2t is also the trailing halo (local row 15) of q-1
                    qprev = qts[(b % 2) * NQ + q - 1]
                    nc.vector.tensor_copy(
                        qprev[:, ct * QCT + 15 * QW + 1 : ct * QCT + 15 * QW + 57],
                        ps2[:, ct * TL : ct * TL + 56],
                    )
                if i == 6 and q < NQ - 1:
                    # row 2t+1 is also the leading halo (local row 0) of q+1
                    qnext = qts[(b % 2) * NQ + q + 1]
                    nc.vector.tensor_copy(
                        qnext[:, ct * QCT + 1 : ct * QCT + 57],
                        ps2[:, ct * TL + 56 : ct * TL + 112],
                    )

        def conv_group(b, q, j):
            qt = qts[(b % 2) * NQ + q]
            for oct in range(2):
                if True:
                    mp = mpsum_pool.tile([128, NG], F32, name="mp", tag="mp")
                    for i in range(18):
                        ct, tap = divmod(i, 9)
                        dh, dw = divmod(tap, 3)
                        q0 = ct * QCT + (7 * j + dh) * QW + dw
                        widx = (ct * 9 + tap) * 2 + oct
                        nc.tensor.matmul(
                            mp[:, :],
                            lhsT=wsb[:, widx * 128 : (widx + 1) * 128],
                            rhs=qt[:, q0 : q0 + NG],
                            start=(i == 0),
                            stop=(i == 17),
                        )
                    ost = outst_pool.tile([128, NG], F32, name="ost", tag="ost")
                    nc.vector.tensor_copy(ost[:, :], mp[:, :])
                    row0 = 14 * q + 7 * j
                    nc.scalar.dma_start(
                        out=out[
                            b, oct * 128 : (oct + 1) * 128, row0 * 56 : row0 * 56 + 392
                        ],
                        in_=ost[:, :].rearrange("p (r x) -> p r x", r=7)[:, :, 0:56],
                    )

        def conv_quarter(b, q):
            conv_group(b, q, 0)
            conv_group(b, q, 1)

        # Quarter q is fully scattered once tile 7q+7 lands; its conv is
        # issued 2 tiles later so the interleaved transposes cover the DVE
        # copy-chain latency at burst start. q3 runs right after the next
        # batch's first tile for the same margin. For batch 0's head, q0 is
        # split into j-halves so PE starts ~4us earlier.
        for b in range(B_LOC):
            for t in range(NT):
                shuffle_tile(b, t)
                if b == 0:
                    if t == 5:
                        conv_group(0, 0, 0)
                    elif t == 9:
                        conv_group(0, 0, 1)
                else:
                    if t == 0:
                        conv_quarter(b - 1, 3)
                    elif t == 9:
                        conv_quarter(b, 0)
                if t == 16:
                    conv_quarter(b, 1)
                elif t == 23:
                    conv_quarter(b, 2)
        conv_quarter(B_LOC - 1, 3)

    nc.compile()
    return nc


def _host_prep(x, w, perm):
    import ml_dtypes

    # pixel-major bf16: [B, HW, C]
    xf = np.ascontiguousarray(
        x.reshape(B, C, HW).transpose(0, 2, 1)
    ).astype(ml_dtypes.bfloat16)

    wt = np.empty((36, 128, 128), dtype=ml_dtypes.bfloat16)
    wf = np.asarray(w, dtype=np.float32)
    for ct in range(2):
        for tap in range(9):
            kh, kw = divmod(tap, 3)
            for oct in range(2):
                i = (ct * 9 + tap) * 2 + oct
                wt[i] = wf[
                    oct * 128 : (oct + 1) * 128, ct * 128 : (ct + 1) * 128, kh, kw
                ].T.astype(ml_dtypes.bfloat16)

    iperm = np.empty((HW, C), dtype=np.int16)
    np.put_along_axis(
        iperm, perm.astype(np.int64), np.arange(C, dtype=np.int16)[None, :], axis=1
    )
    idxt = np.zeros((128, NT * 256), dtype=np.int16)
    for t in range(NT):
        idxt[0:TL, t * 256 : (t + 1) * 256] = iperm[t * TL : t * TL + TL, :]

    in_maps = []
    for cidx in range(N_CORES):
        in_maps.append(
            {
                "xt": np.ascontiguousarray(xf[cidx * B_LOC : (cidx + 1) * B_LOC]),
                "wt": wt,
                "idxt": idxt,
            }
        )
    return in_maps


def kernel(x, w, perm):
    global LAST_RESULT
    _install_ntff_shim()
    from concourse.bass_utils import run_bass_kernel_spmd

    x = np.asarray(x, dtype=np.float32)
    w = np.asarray(w, dtype=np.float32)
    perm = np.asarray(perm)

    if "nc" not in _STATE:
        _STATE["nc"] = _build_kernel()
    nc = _STATE["nc"]

    in_maps = _host_prep(x, w, perm)
    res = run_bass_kernel_spmd(nc, in_maps, core_ids=list(range(N_CORES)))
    LAST_RESULT = res
    out = np.concatenate(
        [r["out"].reshape(B_LOC, C, H, W) for r in res.results], axis=0
    )
    return out.astype(np.float32)
